# revision 16
# baseline (speedup 1.0000x reference)
"""Trainium2 Bass kernel for nn_MultiHeadAttention (B=4, SQ=SK=1024, D=1024,
H=16, DK=DV=64), sharded over 8 NeuronCores as (batch, head-half).

Each core computes one batch's attention for 8 heads:
  - attnsT_half[h, k, q]  (bf16, TRANSPOSED; host casts + transposes)
  - out_partial[s, D]     (f32; host sums the 2 cores of each batch + b_proj)

Layouts on chip (partition dim first):
  qT/kT/vT   : [D, S] (host-pretransposed, bf16)
  qTs/kTs[p] : [128 = dk(h0)|dk(h1), S]   per head-pair p
  v_aug[st]  : [128 s, 4 pairs, 2 heads, 65 = 64 dv | 1 ones]
  scores     : single [k, q] orientation; the two heads of a pair occupy
               disjoint PE row-groups (dk rows 0:64 / 64:128) and their
               K=64 score matmuls are interleaved so they run concurrently.
  softmax    : exp on ScalarE (scale=1/32 folded in, no max subtraction --
               scores are ~N(0, 0.22), max |score/32| < 3, exp safe);
               rowsum comes from the ones-column of v_aug during PV; it is
               broadcast across partitions with a K=1 ones matmul, inverted
               with reciprocal_approx_fast, and multiplied into attnT (the
               attns output) and the PV result (the out path).
"""

import numpy as np
import ml_dtypes

H, D, DK, DV = 16, 1024, 64, 64
B, SQ, SK = 4, 1024, 1024
P = 128
S = 1024
HHALF = 8  # heads per core
NPAIR = 4  # head pairs per core
SCALE = 1.0 / 32.0  # 1/sqrt(D)

_CACHE = {}


def _build_nc():
    from contextlib import ExitStack

    import concourse.tile as tile
    from concourse import bacc, mybir

    BF16 = mybir.dt.bfloat16
    F32 = mybir.dt.float32
    Exp = mybir.ActivationFunctionType.Exp

    nc = bacc.Bacc("TRN2", target_bir_lowering=False, debug=False)

    qT_d = nc.declare_dram_parameter("qT", [D, S], BF16, isOutput=False)
    kT_d = nc.declare_dram_parameter("kT", [D, S], BF16, isOutput=False)
    vT_d = nc.declare_dram_parameter("vT", [D, S], BF16, isOutput=False)
    wq_d = nc.declare_dram_parameter("wq", [D, HHALF * DK], BF16, isOutput=False)
    wk_d = nc.declare_dram_parameter("wk", [D, HHALF * DK], BF16, isOutput=False)
    wv_d = nc.declare_dram_parameter("wv", [D, HHALF * DV], BF16, isOutput=False)
    wp_d = nc.declare_dram_parameter("wprojT", [HHALF * DV, D], BF16, isOutput=False)
    attns_d = nc.declare_dram_parameter("attnsT", [HHALF, SK, SQ], BF16, isOutput=True)
    out_d = nc.declare_dram_parameter("out", [SQ, D], F32, isOutput=True)

    with ExitStack() as ctx:
        tc = ctx.enter_context(tile.TileContext(nc))
        ins = ctx.enter_context(tc.tile_pool(name="ins", bufs=1))
        proj = ctx.enter_context(tc.tile_pool(name="proj", bufs=1))
        work = ctx.enter_context(tc.tile_pool(name="work", bufs=3))
        atp = ctx.enter_context(tc.tile_pool(name="atp", bufs=24))
        norm = ctx.enter_context(tc.tile_pool(name="norm", bufs=2))
        # PSUM budget: 8 banks of [128, 512]f32, split into two pools of
        # 2 x [128,1024] tiles each: psum_s for scores (+ rowsum broadcast),
        # psum_p for projections and PV accumulators. Separate pools let
        # next-pair projections and PV bursts overlap the ACT-paced score
        # loop instead of fighting it for slots.
        psum = ctx.enter_context(tc.tile_pool(name="psum", bufs=2, space="PSUM"))
        psum_p = ctx.enter_context(tc.tile_pool(name="psum_p", bufs=2, space="PSUM"))

        # ---------------- input loads ----------------
        qT_in = ins.tile([P, 8, S], BF16)
        nc.sync.dma_start(out=qT_in, in_=qT_d.rearrange("(c p) s -> p c s", p=P))
        kT_in = ins.tile([P, 8, S], BF16)
        nc.sync.dma_start(out=kT_in, in_=kT_d.rearrange("(c p) s -> p c s", p=P))
        vT_in = ins.tile([P, 8, S], BF16)
        nc.sync.dma_start(out=vT_in, in_=vT_d.rearrange("(c p) s -> p c s", p=P))
        wq_in = ins.tile([P, 8, HHALF * DK], BF16)
        nc.sync.dma_start(out=wq_in, in_=wq_d.rearrange("(c p) m -> p c m", p=P))
        wk_in = ins.tile([P, 8, HHALF * DK], BF16)
        nc.sync.dma_start(out=wk_in, in_=wk_d.rearrange("(c p) m -> p c m", p=P))
        wv_in = ins.tile([P, 8, HHALF * DV], BF16)
        nc.sync.dma_start(out=wv_in, in_=wv_d.rearrange("(c p) m -> p c m", p=P))
        wp_in = ins.tile([P, NPAIR, D], BF16)
        nc.sync.dma_start(out=wp_in, in_=wp_d.rearrange("(c p) m -> p c m", p=P))

        # ---------------- projection / attention op builders ----------------
        # bf16 ones row at partition 64, used to broadcast the PV rowsum
        # (which lands on partition 64 of the PV accumulator) across all 128
        # partitions via a K=1 matmul (compute engines are lane-local).
        ones_t = proj.tile([P, P], BF16, name="ones_t", tag="ones_t")
        nc.vector.memset(ones_t, 1.0)
        outTs = [
            proj.tile([P, S], BF16, name=f"outTs{p}", tag=f"outTs{p}")
            for p in range(NPAIR)
        ]
        qTs = [None] * NPAIR
        kTs = [None] * NPAIR

        def qk_proj_ops(pair):
            """Thunks emitting pair's q/k projections -> [dk-pair, s] tiles.
            qTs[p][0:64, s] = q_proj(head 2p) transposed, [64:128] = head 2p+1.
            q and k run sequentially so only one psum_p 'proj' slot is held."""
            ops = []
            for w_in, src_in, dst_list, nm in (
                (wq_in, qT_in, qTs, "q"),
                (wk_in, kT_in, kTs, "k"),
            ):
                dst = proj.tile(
                    [P, S], BF16, name=f"{nm}Ts{pair}", tag=f"{nm}Ts{pair}"
                )
                dst_list[pair] = dst
                ps = psum_p.tile([P, S], F32, name=f"ps_{nm}{pair}", tag="proj", bufs=1)

                def mk(w_in=w_in, src_in=src_in, ps=ps, pair=pair):
                    def chunk(c):
                        def run():
                            for hf in range(2):
                                nc.tensor.matmul(
                                    ps[:, hf * 512 : (hf + 1) * 512],
                                    lhsT=w_in[:, c, pair * P : (pair + 1) * P],
                                    rhs=src_in[:, c, hf * 512 : (hf + 1) * 512],
                                    start=(c == 0),
                                    stop=(c == 7),
                                )
                        return run
                    return [chunk(c) for c in range(8)]

                ops.extend(mk())
                ops.append(
                    lambda dst=dst, ps=ps: nc.vector.tensor_copy(out=dst, in_=ps)
                )
            return ops

        def pv_norm_ops(pair, atts):
            """Thunks emitting pair's PV + normalization, one q-half at a
            time so a PV accumulator only holds one psum_p 'pv' bank."""
            ops = []
            recips = []
            for sub in range(2):
                rf = norm.tile([P, S], BF16, name="recip_bf", tag="recip_bf")
                recips.append(rf)
            for sub in range(2):
                h = 2 * pair + sub
                recip_bf = recips[sub]
                tmp_o = None
                if sub == 1:
                    tmp_o = work.tile([DV, S], BF16, name="tmp_o", tag="tmp_o")
                for hf in range(2):
                    acc = psum_p.tile([P, 512], F32, name="ps_acc", tag="pv", bufs=2)

                    def pv_half(acc=acc, sub=sub, hf=hf, atts=atts):
                        def run_mm(t):
                            def run():
                                nc.tensor.matmul(
                                    acc[0 : DV + 1, :],
                                    lhsT=v_aug[t][:, pair, sub, :],
                                    rhs=atts[sub][t][:, hf * 512 : (hf + 1) * 512],
                                    start=(t == 0),
                                    stop=(t == 7),
                                )
                            return run
                        return [run_mm(t) for t in range(8)]

                    ops.extend(pv_half())

                    def norm_half(
                        acc=acc, sub=sub, hf=hf, recip_bf=recip_bf, tmp_o=tmp_o
                    ):
                        hsl = slice(hf * 512, (hf + 1) * 512)
                        rsum_bf = norm.tile(
                            [P, 512], BF16, name="rsum_bf", tag="rsum_bf"
                        )
                        nc.vector.tensor_copy(
                            out=rsum_bf[DV : DV + 1, :], in_=acc[DV : DV + 1, :]
                        )
                        ps_bc = psum_p.tile([P, 512], F32, name="ps_bc", tag="pv", bufs=2)
                        nc.tensor.matmul(
                            ps_bc,
                            lhsT=ones_t[DV : DV + 1, :],
                            rhs=rsum_bf[DV : DV + 1, :],
                            start=True,
                            stop=True,
                        )
                        recip_f = norm.tile(
                            [P, 512], F32, name="recip_f", tag="recip_f"
                        )
                        nc.vector.reciprocal_approx_fast(out=recip_f, in_=ps_bc)
                        nc.vector.tensor_copy(out=recip_bf[:, hsl], in_=recip_f)
                        # normalize PV output rows 0:64 = outT (this q-half)
                        if sub == 0:
                            nc.vector.tensor_mul(
                                outTs[pair][0:DV, hsl],
                                acc[0:DV, :],
                                recip_f[0:DV, :],
                            )
                        else:
                            nc.vector.tensor_mul(
                                tmp_o[:, hsl], acc[0:DV, :], recip_f[0:DV, :]
                            )

                    ops.append(norm_half)

                # normalize attnT in place (bf16 x bf16 -> 2x DVE) and store
                def at_norm(sub=sub, h=h, recip_bf=recip_bf, atts=atts):
                    def run_t(t):
                        def run():
                            at = atts[sub][t]
                            nc.vector.tensor_mul(at, at, recip_bf)
                            nc.sync.dma_start(
                                out=attns_d[h, t * P : (t + 1) * P, :], in_=at
                            )
                        return run
                    return [run_t(t) for t in range(8)]

                ops.extend(at_norm())
                if sub == 1:
                    ops.append(
                        lambda tmp_o=tmp_o, pair=pair: nc.gpsimd.dma_start(
                            out=outTs[pair][DV:P, :], in_=tmp_o
                        )
                    )
            return ops

        # ---------------- v projection -> [s, head, dv | ones] ----------------
        # v_aug[st] : [128, 4 pairs, 2 heads, 65]; col 64 of each head is 1.0
        v_aug = []
        for st in range(8):
            va = proj.tile([P, NPAIR, 2, DV + 1], BF16, name=f"v_aug{st}", tag=f"v_aug{st}")
            ps = psum_p.tile([P, HHALF * DV], F32, name=f"ps_v{st}", tag="proj", bufs=1)
            for c in range(8):
                nc.tensor.matmul(
                    ps,
                    lhsT=vT_in[:, c, st * P : (st + 1) * P],
                    rhs=wv_in[:, c, :],
                    start=(c == 0),
                    stop=(c == 7),
                )
            ps_v = ps.rearrange("p (pr two d) -> p pr two d", two=2, d=DV)
            nc.vector.tensor_copy(out=va[:, :, :, 0:DV], in_=ps_v)
            nc.vector.memset(va[:, :, :, DV : DV + 1], 1.0)
            v_aug.append(va)

        # pair 0's q/k projections run upfront
        for op in qk_proj_ops(0):
            op()

        # ---------------- attention: score loops with interleaved background --
        # Window p: ACT-paced score loop for pair p, with pair p-1's PV +
        # normalization and pair p+1's q/k projections threaded through it to
        # keep the PE array dense (HAM clock-gate stays warm).
        prev_atts = None
        for pair in range(NPAIR):
            bg = []
            if prev_atts is not None:
                bg.extend(pv_norm_ops(pair - 1, prev_atts))
            if pair + 1 < NPAIR:
                bg.extend(qk_proj_ops(pair + 1))
            per_t = (len(bg) + 7) // 8

            atts = [[], []]
            for t in range(8):
                pss = [
                    psum.tile([P, S], F32, name=f"ps_s{sub}", tag="ps_big")
                    for sub in range(2)
                ]
                # Interleave the two heads' K=64 score matmuls: disjoint PE
                # row-groups (dk rows 0:64 vs 64:128) -> concurrent execution.
                for hf in range(2):
                    for sub in range(2):
                        hsl = slice(64 * sub, 64 * sub + 64)
                        nc.tensor.matmul(
                            pss[sub][:, hf * 512 : (hf + 1) * 512],
                            lhsT=kTs[pair][hsl, t * P : (t + 1) * P],
                            rhs=qTs[pair][hsl, hf * 512 : (hf + 1) * 512],
                            start=True,
                            stop=True,
                        )
                for sub in range(2):
                    at = atp.tile([P, S], BF16, name="attnT", tag="attnT")
                    nc.scalar.activation(out=at, in_=pss[sub], func=Exp, scale=SCALE)
                    atts[sub].append(at)
                for op in bg[t * per_t : (t + 1) * per_t]:
                    op()
            for op in bg[8 * per_t :]:
                op()
            prev_atts = atts

        # last pair's PV + normalization
        for op in pv_norm_ops(NPAIR - 1, prev_atts):
            op()

        # ---------------- output projection ----------------
        for st in range(8):
            ps_o = psum.tile([P, S], F32, name="ps_o", tag="ps_big")
            for pc in range(NPAIR):
                for hf in range(2):
                    nc.tensor.matmul(
                        ps_o[:, hf * 512 : (hf + 1) * 512],
                        lhsT=outTs[pc][:, st * P : (st + 1) * P],
                        rhs=wp_in[:, pc, hf * 512 : (hf + 1) * 512],
                        start=(pc == 0),
                        stop=(pc == NPAIR - 1),
                    )
            out_sb = work.tile([P, S], F32, name="out_sb", tag="out_sb")
            nc.vector.tensor_copy(out=out_sb, in_=ps_o)
            nc.sync.dma_start(out=out_d[st * P : (st + 1) * P, :], in_=out_sb)

    nc.compile()
    return nc


def _get_nc():
    if "nc" not in _CACHE:
        _CACHE["nc"] = _build_nc()
    return _CACHE["nc"]


def _prep_in_maps(query, key, value, w_q, w_k, w_v, w_proj):
    bf = ml_dtypes.bfloat16
    per_batch = []
    for b in range(B):
        per_batch.append(
            (
                np.ascontiguousarray(query[b].T).astype(bf),
                np.ascontiguousarray(key[b].T).astype(bf),
                np.ascontiguousarray(value[b].T).astype(bf),
            )
        )
    per_half = []
    for half in range(2):
        h0 = half * HHALF
        wq = np.ascontiguousarray(
            w_q[h0 : h0 + HHALF].transpose(1, 0, 2).reshape(D, HHALF * DK)
        ).astype(bf)
        wk = np.ascontiguousarray(
            w_k[h0 : h0 + HHALF].transpose(1, 0, 2).reshape(D, HHALF * DK)
        ).astype(bf)
        wv = np.ascontiguousarray(
            w_v[h0 : h0 + HHALF].transpose(1, 0, 2).reshape(D, HHALF * DV)
        ).astype(bf)
        wpT = np.ascontiguousarray(
            w_proj[:, DV * h0 : DV * (h0 + HHALF)].T
        ).astype(bf)
        per_half.append((wq, wk, wv, wpT))

    in_maps = []
    for core in range(8):
        b, half = divmod(core, 2)
        qT, kT, vT = per_batch[b]
        wq, wk, wv, wpT = per_half[half]
        in_maps.append(
            {"qT": qT, "kT": kT, "vT": vT, "wq": wq, "wk": wk, "wv": wv, "wprojT": wpT}
        )
    return in_maps


def _run(in_maps, trace=False):
    from concourse.bass_utils import run_bass_kernel_spmd

    return run_bass_kernel_spmd(
        _get_nc(), in_maps, core_ids=list(range(8)), trace=trace
    )


def _gather(results, b_proj):
    out = np.zeros((B, SQ, D), np.float32)
    attns = np.empty((H * B, SQ, SK), np.float32)
    for core in range(8):
        b, half = divmod(core, 2)
        h0 = half * HHALF
        out[b] += results[core]["out"]
        ahT = results[core]["attnsT"]  # [HHALF, SK, SQ] bf16
        for i in range(HHALF):
            attns[(h0 + i) * B + b] = ahT[i].T.astype(np.float32)
    out += b_proj
    return out, attns


def kernel(query, key, value, w_q, w_k, w_v, w_proj, b_proj):
    query = np.asarray(query, dtype=np.float32)
    key = np.asarray(key, dtype=np.float32)
    value = np.asarray(value, dtype=np.float32)
    w_q = np.asarray(w_q, dtype=np.float32)
    w_k = np.asarray(w_k, dtype=np.float32)
    w_v = np.asarray(w_v, dtype=np.float32)
    w_proj = np.asarray(w_proj, dtype=np.float32)
    b_proj = np.asarray(b_proj, dtype=np.float32)

    in_maps = _prep_in_maps(query, key, value, w_q, w_k, w_v, w_proj)
    results = _run(in_maps).results
    return _gather(results, b_proj)
